# revision 1
# baseline (speedup 1.0000x reference)
"""Causal single-head attention (B=4, S=4096, D=1024, H=64) on 8 TRN2 NeuronCores.

Strategy
--------
Data-parallel over the batch (2 cores per batch element); within each pair the
k/v work is split by 128-row k-block parity and the q/x loading is split the
same way, so each core DMAs only HALF its batch's activations (8 MB).  Each
core projects q/k/v for its own row blocks, the pair exchanges q^T halves with
small pipelined AllReduces (each core's absent half is zeros, so add ==
exchange), and each core then computes the partial unnormalized attention
  out_u = sum_{k in my blocks} exp(q.k/8 + mask) * [v | 1]
for ALL 4096 q rows.  The pairs' partials are summed with pipelined
AllReduces and normalized on device (divide by the exp-sum row that falls out
of the ones column of v').  The causal work per core is exactly half the
triangle regardless of parity, so the 8 cores are perfectly load balanced.

The SPMD program is identical on all 8 cores.  Per-core differences are pure
data: the xT shard contents, host-computed causal mask tiles, and a few
partition_id-derived dynamic copy offsets (CC staging slots and the canonical
[even blocks | odd blocks] partial layout the pair's collectives add over).
Attention is split by q-column half: each super-tile's own-parity columns are
attended straight off the local projections (no communication on that path),
while the peer-parity columns trail the q exchange — so the collectives sit
entirely off the critical path.

On-chip dataflow: projections contract D on the partition axis (host supplies
x pre-transposed, a layout-only prep), producing qT/kT [H, S-blocks] for the
scores matmul  scoresT = kT_blk.T @ qT_cols,  while v is re-transposed on the
tensor engine to natural [Sk, H+1] (ones column appended) so that
  out_uT = v'.T @ exp(scoresT)
accumulates over k blocks in PSUM.  exp() runs on the scalar engine straight
out of PSUM with the 1/sqrt(H) scale folded in; no row-max subtraction is
needed for these inputs (|scores|/8 <~ 6) and masked entries underflow to
exactly 0, matching the reference's -1e9 semantics.  Matmuls use float32r
(full-rate PE streaming, ~3e-4 scale-relative output error); set MM_DT = F32
for full fp32 (4x slower matmuls, ~7e-6 error).  Projection chunks, q
exchanges, attention super-tiles, and combine/normalize pieces are emitted
interleaved so DMA (sync ring: activations; scalar ring: weights/exchange),
PE, ACT, DVE, and the collectives all pipeline.

The host only does layout work (transpose/permute/slice/gather); every FLOP
of the module runs on device.
"""

import numpy as np
from contextlib import ExitStack

import concourse.bass as bass
import concourse.mybir as mybir
import concourse.tile as tile
from concourse import bacc
from concourse.bass_utils import run_bass_kernel_spmd
from concourse.masks import make_identity

F32 = mybir.dt.float32
F32R = mybir.dt.float32r

B, S, D, H = 4, 4096, 1024, 64
NCORES = 8
SQT = 512          # q super-tile width
NST = S // SQT     # 8 super-tiles
NCH = D // 128     # 8 contraction chunks
NKB = S // 128     # 32 k blocks per batch
MASK_VAL = -8.0e9  # added to raw scores; exp(0.125*(s+MASK_VAL)) == 0
SCALE = 0.125      # 1/sqrt(H)

# matmul compute dtype: F32R streams 4x faster on the PE at slightly reduced
# multiply precision; flip to F32 for full precision.
MM_DT = F32R


def build_program(with_cc: bool = True):
    nc = bacc.Bacc(num_devices=NCORES)

    xT = nc.declare_dram_parameter("xT", [D, S // 2], MM_DT, isOutput=False)
    wq = nc.declare_dram_parameter("wq", [D, H], MM_DT, isOutput=False)
    wk = nc.declare_dram_parameter("wk", [D, H], MM_DT, isOutput=False)
    wv = nc.declare_dram_parameter("wv", [D, H], MM_DT, isOutput=False)
    bq = nc.declare_dram_parameter("bq", [H, 1], F32, isOutput=False)
    bk = nc.declare_dram_parameter("bk", [H, 1], F32, isOutput=False)
    bv = nc.declare_dram_parameter("bv", [H, 1], F32, isOutput=False)
    masks = nc.declare_dram_parameter("masks", [2, 128, SQT], F32, isOutput=False)
    out = nc.declare_dram_parameter("out", [H, S], F32, isOutput=True)

    xT3 = xT.rearrange("(c p) s -> p c s", p=128)      # [128, 8, 2048]
    wq3 = wq.rearrange("(c p) h -> p c h", p=128)      # [128, 8, 64]
    wk3 = wk.rearrange("(c p) h -> p c h", p=128)
    wv3 = wv.rearrange("(c p) h -> p c h", p=128)
    masks3 = masks.rearrange("m p j -> p m j")          # [128, 2, 512]

    with ExitStack() as ctx:
        tc = ctx.enter_context(tile.TileContext(nc))

        pid = nc.partition_id()
        parity = pid % 2
        off_my = parity * 2048           # my blocks' canonical half (partial)
        off_ot = ((pid + 1) % 2) * 2048  # peer blocks' canonical half
        slot_my = parity * 1024          # my slot inside a CC staging piece
        slot_ot = ((pid + 1) % 2) * 1024

        singles = ctx.enter_context(tc.tile_pool(name="singles", bufs=1))
        dram = ctx.enter_context(tc.tile_pool(name="dram", bufs=1, space="DRAM"))

        wq_sb = singles.tile([128, NCH, H], MM_DT)
        wkv_sb = singles.tile([128, NCH, 2 * H], MM_DT)  # [Wk | Wv] packed
        nc.scalar.dma_start(out=wq_sb, in_=wq3)
        nc.scalar.dma_start(out=wkv_sb[:, :, 0:H], in_=wk3)
        nc.scalar.dma_start(out=wkv_sb[:, :, H : 2 * H], in_=wv3)
        bq_sb = singles.tile([H, 1], F32)
        bk_sb = singles.tile([H, 1], F32)
        bv_sb = singles.tile([2 * H, 1], F32)  # rows 64..127 hold bv
        nc.scalar.dma_start(out=bq_sb, in_=bq[:, :])
        nc.scalar.dma_start(out=bk_sb, in_=bk[:, :])
        nc.scalar.dma_start(out=bv_sb[H : 2 * H, :], in_=bv[:, :])
        masks_sb = singles.tile([128, 2, SQT], F32)
        nc.gpsimd.dma_start(out=masks_sb, in_=masks3)

        ident = singles.tile([2 * H, H], F32)
        make_identity(nc, ident[0:H, :])
        nc.scalar.dma_start(out=ident[H : 2 * H, :], in_=ident[0:H, :])

        qT_perm = singles.tile([H, S // 2], MM_DT)  # my q blocks (block-packed)
        qT_ot = singles.tile([H, S // 2], MM_DT)    # peer q blocks (from exchange)
        qex = singles.tile([H, S // 2], F32, name="qex")    # CC staging out
        qex2 = singles.tile([H, S // 2], F32, name="qex2")  # CC staging in
        nc.vector.memset(qex, 0.0)  # absent slots stay 0 so CC add == exchange
        kT_sb = singles.tile([H, 16, 128], MM_DT)    # my 16 k blocks
        vT_sb = singles.tile([2 * H, 16, 128], F32)  # v^T in rows 64..127
        # v' natural [Sk, 65]: cols 0..63 = v, col 64 = ones (exp-sum row)
        v_sb = singles.tile([128, 16, H + 1], MM_DT)
        partial = singles.tile([H + 1, S], F32)  # unnormalized out^T + sums
        nc.vector.memset(v_sb[:, :, H : H + 1].bitcast(F32), 1.0)

        COMB = [(0, 1024), (1024, 512), (1536, 256), (1792, 256)]
        cc_in = [
            dram.tile([H + 1, 2 * n], F32, tag=f"ci{p}", name=f"cc_in{p}")
            for p, (_, n) in enumerate(COMB)
        ]
        cc_red = [
            dram.tile([H + 1, 2 * n], F32, tag=f"cr{p}", name=f"cc_red{p}")
            for p, (_, n) in enumerate(COMB)
        ]
        qcc_in = [
            dram.tile([H, S // 2], F32, tag=f"qi{p}", name=f"qcc_in{p}")
            for p in range(2)
        ]
        qcc_red = [
            dram.tile([H, S // 2], F32, tag=f"qr{p}", name=f"qcc_red{p}")
            for p in range(2)
        ]

        xpool = ctx.enter_context(tc.tile_pool(name="xt", bufs=4))
        pj = ctx.enter_context(tc.tile_pool(name="pj", bufs=2, space="PSUM"))
        ps_pool = ctx.enter_context(tc.tile_pool(name="ps", bufs=4, space="PSUM"))
        pu_pool = ctx.enter_context(tc.tile_pool(name="pu", bufs=1, space="PSUM"))
        pvt = ctx.enter_context(tc.tile_pool(name="pvt", bufs=1, space="PSUM"))
        pexp_pool = ctx.enter_context(tc.tile_pool(name="pexp", bufs=4))

        def proj_chunk(i):
            """DMA my-half xT chunk i (i<4) and project q/k/v."""
            xt = xpool.tile([128, NCH, SQT], MM_DT)
            src_i = xT3[:, :, i * SQT : (i + 1) * SQT]
            nc.sync.dma_start(out=xt[:, 0 : NCH // 2, :], in_=src_i[:, 0 : NCH // 2, :])
            nc.sync.dma_start(out=xt[:, NCH // 2 :, :], in_=src_i[:, NCH // 2 :, :])

            psq = pj.tile([H, SQT], F32, tag="pj")
            for c in range(NCH):
                nc.tensor.matmul(
                    psq, lhsT=wq_sb[:, c, :], rhs=xt[:, c, :],
                    start=(c == 0), stop=(c == NCH - 1),
                )
            nc.vector.tensor_scalar_add(
                qT_perm[:, i * SQT : (i + 1) * SQT], psq, bq_sb
            )

            pskv = pj.tile([128, SQT], F32, tag="pj")
            for c in range(NCH):
                nc.tensor.matmul(
                    pskv, lhsT=wkv_sb[:, c, :], rhs=xt[:, c, :],
                    start=(c == 0), stop=(c == NCH - 1),
                )
            nc.vector.tensor_scalar_add(
                kT_sb[:, 4 * i : 4 * i + 4, :], pskv[0:H, :], bk_sb
            )
            nc.vector.tensor_scalar_add(
                vT_sb[H : 2 * H, 4 * i : 4 * i + 4, :],
                pskv[H : 2 * H, :],
                bv_sb[H : 2 * H, :],
            )
            # v natural blocks via PE transpose (from partition group 64..127)
            for jb in range(4 * i, 4 * i + 4):
                pvt_t = pvt.tile([128, H], F32)
                nc.tensor.transpose(
                    pvt_t, vT_sb[H : 2 * H, jb, :], ident[H : 2 * H, :]
                )
                nc.vector.tensor_copy(v_sb[:, jb, 0:H], pvt_t)

        def q_exchange(p):
            """Exchange q blocks 8p..8p+7 with the pair peer.

            Stages my 1024 columns into my slot of a zero-initialized buffer
            (peer slot stays 0), pair-AllReduce-adds, and copies the peer's
            slot into qT_ot.  qT_ot is a separate tensor so the dynamic
            writes never serialize against my-half attention reads.
            """
            lo = 1024 * p
            nc.vector.tensor_copy(
                qex[:, bass.ds(slot_my, 1024)].bitcast(MM_DT),
                qT_perm[:, lo : lo + 1024],
            )
            nc.scalar.dma_start(out=qcc_in[p][:, :], in_=qex)
            if with_cc:
                nc.gpsimd.collective_compute(
                    "AllReduce",
                    mybir.AluOpType.add,
                    replica_groups=[[0, 1], [2, 3], [4, 5], [6, 7]],
                    ins=[qcc_in[p][:, :]],
                    outs=[qcc_red[p][:, :]],
                )
            else:
                nc.gpsimd.dma_start(out=qcc_red[p][:, :], in_=qcc_in[p][:, :])
            nc.scalar.dma_start(out=qex2, in_=qcc_red[p][:, :])
            nc.vector.tensor_copy(
                qT_ot[:, lo : lo + 1024],
                qex2[:, bass.ds(slot_ot, 1024)].bitcast(MM_DT),
            )

        def attention_st(st, half):
            """Attention for super-tile st, one 256-col q half.

            half 0 = my-parity q blocks {2st, 2st+1} (no exchange needed),
            half 1 = peer blocks (reads qT_ot after the exchange).
            """
            qsrc = qT_perm if half == 0 else qT_ot
            q_rhs = qsrc[:, 256 * st : 256 * st + 256]  # [64, 256]
            out_u = pu_pool.tile([H + 1, 256], F32, tag="out_u")
            nj = 2 * st + 2
            for jp in range(st + 1):  # pairs of k blocks
                ps2 = ps_pool.tile([128, SQT], F32, tag="ps")
                for h2 in range(2):
                    j = 2 * jp + h2
                    nc.tensor.matmul(
                        ps2[:, h2 * 256 : (h2 + 1) * 256],
                        lhsT=kT_sb[:, j, :], rhs=q_rhs,
                        start=True, stop=True,
                    )
                if jp == st:  # causal band: mask both blocks in one op
                    nc.vector.tensor_add(
                        ps2,
                        ps2,
                        masks_sb[:, :, 256 * half : 256 * half + 256],
                    )
                pexp = pexp_pool.tile([128, SQT], MM_DT, tag="pexp")
                nc.scalar.activation(
                    pexp, ps2, mybir.ActivationFunctionType.Exp, scale=SCALE
                )
                for h2 in range(2):
                    j = 2 * jp + h2
                    nc.tensor.matmul(
                        out_u,
                        lhsT=v_sb[:, j, :],
                        rhs=pexp[:, h2 * 256 : (h2 + 1) * 256],
                        start=(j == 0), stop=(j == nj - 1),
                    )
            off_h = off_my if half == 0 else off_ot
            nc.vector.tensor_copy(
                partial[:, bass.ds(off_h + 256 * st, 256)], out_u
            )

        sumv = [
            singles.tile([H, 2 * n], F32, name=f"sumv{p}")
            for p, (_, n) in enumerate(COMB)
        ]
        srep = [
            singles.tile([H, 2 * n], F32, name=f"srep{p}")
            for p, (_, n) in enumerate(COMB)
        ]
        ostage = [
            singles.tile([H, 2 * n], F32, name=f"ostage{p}")
            for p, (_, n) in enumerate(COMB)
        ]
        pruns = partial.rearrange("h (half c) -> h half c", half=2)
        oruns = out.rearrange("h (half c) -> h half c", half=2)

        def combine_piece(rho):
            """Pair-AllReduce + normalize of canonical columns piece rho."""
            lo, n = COMB[rho]
            src = pruns[:, :, lo : lo + n]  # [65, 2, n]
            nc.sync.dma_start(out=cc_in[rho][:, :], in_=src)
            if with_cc:
                nc.gpsimd.collective_compute(
                    "AllReduce",
                    mybir.AluOpType.add,
                    replica_groups=[[0, 1], [2, 3], [4, 5], [6, 7]],
                    ins=[cc_in[rho][:, :]],
                    outs=[cc_red[rho][:, :]],
                )
            else:
                nc.gpsimd.dma_start(out=cc_red[rho][:, :], in_=cc_in[rho][:, :])
            nc.gpsimd.dma_start(out=sumv[rho], in_=cc_red[rho][0 : H, :])
            srow_b = bass.AP(
                tensor=cc_red[rho][:, :].tensor,
                offset=cc_red[rho][:, :].offset + H * 2 * n,
                ap=[[0, H], [1, 2 * n]],
            )
            nc.scalar.dma_start(out=srep[rho], in_=srow_b)
            nc.vector.reciprocal(srep[rho], srep[rho])
            nc.vector.tensor_mul(ostage[rho], sumv[rho], srep[rho])
            nc.scalar.dma_start(out=oruns[:, :, lo : lo + n], in_=ostage[rho])

        # pipelined emission: my-half attention starts right behind the
        # projections; the q exchange and the peer-half attention trail it;
        # combine pieces run behind the attention wave
        proj_chunk(0)
        attention_st(0, 0)
        attention_st(1, 0)
        proj_chunk(1)
        attention_st(2, 0)
        attention_st(3, 0)
        q_exchange(0)
        proj_chunk(2)
        attention_st(4, 0)
        attention_st(5, 0)
        attention_st(0, 1)
        attention_st(1, 1)
        proj_chunk(3)
        q_exchange(1)
        attention_st(2, 1)
        attention_st(3, 1)
        attention_st(6, 0)
        attention_st(7, 0)
        attention_st(4, 1)
        attention_st(7, 1)
        combine_piece(0)
        attention_st(5, 1)
        combine_piece(1)
        attention_st(6, 1)
        combine_piece(3)
        combine_piece(2)

        # (combine pieces are emitted inline above)

    nc.finalize()
    return nc


def _make_masks(parity: int) -> np.ndarray:
    """Two [128, 512] additive mask tiles for the causal-band k-block pair.

    scores^T tile for super-tile st, band block j = 2*st + jrel:
    rows p = k rows of block kb = 4*st + 2*jrel + parity; columns
    cols 256*half + 128*h2 with (half, h2) -> natural q block 4*st + NU[sc],
    sc = 2*half + h2, NU = [parity, 2+parity, 1-parity, 3-parity].
    """
    NU = [parity, 2 + parity, 1 - parity, 3 - parity]
    m = np.zeros((2, 128, SQT), np.float32)
    p = np.arange(128)
    jc = np.arange(128)
    for jrel in range(2):
        rho = 2 * jrel + parity
        for sc in range(4):
            q_rel = 128 * NU[sc] + jc[None, :]
            k_rel = 128 * rho + p[:, None]
            m[jrel][:, 128 * sc : 128 * (sc + 1)] = np.where(
                q_rel < k_rel, MASK_VAL, 0.0
            ).astype(np.float32)
    return m


_PROGRAM_CACHE = {}


def _get_program():
    key = "prog"
    if key not in _PROGRAM_CACHE:
        _PROGRAM_CACHE[key] = build_program()
    return _PROGRAM_CACHE[key]


def kernel(x, Wq, bq, Wk, bk, Wv, bv):
    x = np.asarray(x, dtype=np.float32)
    Wq = np.asarray(Wq, dtype=np.float32)
    Wk = np.asarray(Wk, dtype=np.float32)
    Wv = np.asarray(Wv, dtype=np.float32)
    bq = np.asarray(bq, dtype=np.float32).reshape(H, 1)
    bk = np.asarray(bk, dtype=np.float32).reshape(H, 1)
    bv = np.asarray(bv, dtype=np.float32).reshape(H, 1)

    nc = _get_program()

    in_maps = []
    for core in range(NCORES):
        b, parity = core // 2, core % 2
        xb = x[b]  # [S, D]
        # this core's shard: its parity's 128-row blocks only (half the rows)
        blocks = list(range(parity, NKB, 2))
        xTp = np.ascontiguousarray(
            xb.T.reshape(D, NKB, 128)[:, blocks, :].reshape(D, S // 2)
        )
        in_maps.append(
            {
                "xT": xTp,
                "wq": Wq,
                "wk": Wk,
                "wv": Wv,
                "bq": bq,
                "bk": bk,
                "bv": bv,
                "masks": _make_masks(parity),
            }
        )

    res = run_bass_kernel_spmd(nc, in_maps, list(range(NCORES)))

    # canonical column order -> natural, transpose back
    # canonical layout: [even natural blocks | odd natural blocks];
    # natural block n sits at canonical position n//2 + 16*(n%2)
    order = [(n // 2) + 16 * (n % 2) for n in range(NKB)]
    out = np.empty((B, S, H), np.float32)
    for b in range(B):
        oT = res.results[2 * b]["out"]  # [64, 4096] canonical
        oT_nat = oT.reshape(H, NKB, 128)[:, order, :].reshape(H, S)
        out[b] = oT_nat.T
    return out

